# revision 15
# baseline (speedup 1.0000x reference)
"""Causal self-attention Trainium2 kernel.

Reference (full): x[B=2,S=2048,D=1024] @ W_qkv + b_qkv -> 16-head causal
attention -> @ W_out + b_out.

Sharding: 8 cores = (batch b in 0..1) x (head-group hg in 0..3, 4 heads of
hd=64 each). Each core computes a partial output projection for its 4 heads
on its batch; the host sums the 4 head-group partials per batch and adds the
(constant) V-bias correction bv @ W_out and b_out.

Device pipeline per core (fp32r matmuls == fp32 bitwise on TRN2 PE, 4x faster):
  1. x -> xT via PE transposes.
  2. QK^T projection: qkt[c, s] = (W_qk.T @ x.T), bias added on eviction.
  3. V projection in natural layout v[s, c] with a ones column appended
     (the ones column produces the softmax denominator during PV).
  4. Per (head, q-span): scores transposed ST[k, q] = KT.T-slice @ QT-slice,
     exp on ACT (scale=1/8), triangle mask on diagonal blocks only,
     PV accumulate attnT[c, q] (+ denominator row).  Causality by skipping
     fully-masked column ranges (partial-N matmuls).
  5. Normalize: reciprocal of denominator row, broadcast across partitions
     via a one-hot selector matmul on the PE, multiply on eviction to attnT.
  6. Output projection y[s, o] from attnT and W_out rows; DMA out.
"""
import numpy as np
from contextlib import ExitStack

import concourse.bacc as bacc
import concourse.tile as tile
from concourse import mybir
from concourse.bass_utils import run_bass_kernel_spmd

F32 = mybir.dt.float32
F32R = mybir.dt.float32r

B = 2
S = 2048
D = 1024
NH = 16
HD = 64
HG = 4            # head-groups (cores per batch)
HPG = 4           # heads per group
CL = HPG * HD     # 256 local head cols per core
P = 128
NSC = S // P      # 16 s-chunks
NDC = D // P      # 8 d-chunks
NQJ = S // 512    # 4 q-spans
NKC = S // P      # 16 k-chunks

_CACHED = {}


def _build():
    if "nc" in _CACHED:
        return _CACHED["nc"]
    nc = bacc.Bacc("TRN2", target_bir_lowering=False, debug=False)

    x_d = nc.dram_tensor("x", [S, D], F32, kind="ExternalInput")
    wqk_d = nc.dram_tensor("wqk", [D, 2 * CL], F32, kind="ExternalInput")
    wv_d = nc.dram_tensor("wv", [D, CL], F32, kind="ExternalInput")
    wout_d = nc.dram_tensor("wout", [CL, D], F32, kind="ExternalInput")
    bqk_d = nc.dram_tensor("bqk", [P, 4], F32, kind="ExternalInput")
    tri_d = nc.dram_tensor("tri", [P, P], F32, kind="ExternalInput")
    idn_d = nc.dram_tensor("idn", [P, P], F32, kind="ExternalInput")
    ones_d = nc.dram_tensor("ones", [P, 68], F32, kind="ExternalInput")
    y_d = nc.dram_tensor("y", [S, D], F32, kind="ExternalOutput")

    with tile.TileContext(nc) as tc, ExitStack() as ctx:
        persist = ctx.enter_context(tc.tile_pool(name="persist", bufs=1))
        ptp = ctx.enter_context(tc.tile_pool(name="ptp", bufs=3))
        youtp = ctx.enter_context(tc.tile_pool(name="youtp", bufs=2))
        rcpp = ctx.enter_context(tc.tile_pool(name="rcpp", bufs=3))
        tmpp = ctx.enter_context(tc.tile_pool(name="tmpp", bufs=2))
        ps_sm = ctx.enter_context(tc.tile_pool(name="ps_sm", bufs=2, space="PSUM"))
        ps_st = ctx.enter_context(tc.tile_pool(name="ps_st", bufs=2, space="PSUM"))
        ps_av = ctx.enter_context(tc.tile_pool(name="ps_av", bufs=2, space="PSUM"))

        # ---- persistent tiles ----
        qkt_sb = persist.tile([P, 4, S], F32R, name="qkt_sb")       # 32KB/part
        v_sb = persist.tile([P, NKC, HPG, HD + 1], F32R, name="v_sb")
        attnT = persist.tile([P, 2, S], F32R, name="attnT")         # 16KB/part
        wout_sb = persist.tile([P, 2, D], F32R, name="wout_sb")
        bqk_sb = persist.tile([P, 4], F32, name="bqk_sb")
        tri_sb = persist.tile([P, P], F32R, name="tri_sb")
        ident = persist.tile([P, P], F32, name="ident")
        ones_sb = persist.tile([P, 68], F32R, name="ones_sb")

        nc.sync.dma_start(out=wout_sb, in_=wout_d.ap().bitcast(F32R)
                          .rearrange("(c p) o -> p c o", p=P))
        nc.sync.dma_start(out=bqk_sb, in_=bqk_d.ap())
        nc.sync.dma_start(out=tri_sb, in_=tri_d.ap().bitcast(F32R))
        nc.sync.dma_start(out=ident, in_=idn_d.ap())
        nc.sync.dma_start(out=ones_sb, in_=ones_d.ap().bitcast(F32R))
        ones_row = ones_sb[0:1, 4:4 + HD]

        # fill ones column of V tiles from the DMA'd ones tile
        for sc in range(NSC):
            nc.vector.tensor_copy(v_sb[:, sc, :, HD], ones_sb[:, 0:4])

        # ---- phase 1+2: load x, transpose to xT ----
        with tc.tile_pool(name="xTp", bufs=1) as xTp:
            xT = xTp.tile([P, NDC, S], F32R, name="xT")             # 64KB/part
            with tc.tile_pool(name="xp", bufs=5) as xp:
                for scq in range(4):
                    xts = []
                    for si in range(4):
                        sc = scq * 4 + si
                        xt = xp.tile([P, D], F32, tag="x", name=f"x{sc}")
                        nc.sync.dma_start(out=xt, in_=x_d.ap()[sc * P:(sc + 1) * P, :])
                        xts.append(xt)
                    for dc in range(NDC):
                        ptr = ps_sm.tile([P, 512], F32, tag="sm", name=f"ptr{scq}_{dc}")
                        for si in range(4):
                            nc.tensor.transpose(
                                ptr[:, si * P:(si + 1) * P],
                                xts[si][:, dc * P:(dc + 1) * P],
                                ident)
                        nc.vector.tensor_copy(
                            xT[:, dc, scq * 512:(scq + 1) * 512],
                            ptr.bitcast(F32R))

            # ---- phase 3: QK^T projection ----
            with tc.tile_pool(name="wp", bufs=1) as wp:
                wqk_sb = wp.tile([P, NDC, 2 * CL], F32R, name="wqk_sb")
                wv_sb = wp.tile([P, NDC, CL], F32R, name="wv_sb")
                nc.sync.dma_start(out=wqk_sb, in_=wqk_d.ap().bitcast(F32R)
                                  .rearrange("(c p) m -> p c m", p=P))
                nc.sync.dma_start(out=wv_sb, in_=wv_d.ap().bitcast(F32R)
                                  .rearrange("(c p) m -> p c m", p=P))

                for mc in range(4):
                    for ns in range(4):
                        pq = ps_sm.tile([P, 512], F32, tag="sm", name=f"pq{mc}_{ns}")
                        for kc in range(NDC):
                            nc.tensor.matmul(
                                pq[:],
                                wqk_sb[:, kc, mc * P:(mc + 1) * P],
                                xT[:, kc, ns * 512:(ns + 1) * 512],
                                start=(kc == 0), stop=(kc == NDC - 1))
                        nc.vector.tensor_scalar_add(
                            qkt_sb[:, mc, ns * 512:(ns + 1) * 512],
                            pq[:], bqk_sb[:, mc:mc + 1])

                # ---- phase 4: V projection (natural layout) ----
                for sc in range(NSC):
                    pv = ps_sm.tile([P, CL], F32, tag="sm", name=f"pv{sc}")
                    for kc in range(NDC):
                        nc.tensor.matmul(
                            pv[:],
                            xT[:, kc, sc * P:(sc + 1) * P],
                            wv_sb[:, kc, :],
                            start=(kc == 0), stop=(kc == NDC - 1))
                    nc.vector.tensor_copy(
                        v_sb[:, sc, :, 0:HD],
                        pv.rearrange("p (h d) -> p h d", h=HPG))

        # ---- phase 5: attention per (q-span, head) ----
        for qj in range(NQJ):
            q0 = qj * 512
            nkc = 4 * (qj + 1)
            for h in range(HPG):
                mck, pok = 2 + h // 2, 64 * (h % 2)
                mcq, poq = h // 2, 64 * (h % 2)
                av = ps_av.tile([P, 512], F32, tag="av", name=f"av{qj}_{h}")
                for pi in range(nkc // 2):
                    stp = ps_st.tile([P, 1024], F32, tag="st", name=f"st{qj}_{h}_{pi}")
                    pt = ptp.tile([P, 1024], F32R, tag="pt", name=f"pt{qj}_{h}_{pi}")
                    for half in range(2):
                        kc = 2 * pi + half
                        t = kc - 4 * qj
                        c0 = 128 * t if t > 0 else 0
                        nc.tensor.matmul(
                            stp[:, 512 * half + c0: 512 * half + 512],
                            qkt_sb[pok:pok + 64, mck, kc * P:(kc + 1) * P],
                            qkt_sb[poq:poq + 64, mcq, q0 + c0: q0 + 512],
                            start=True, stop=True)
                    t0 = 2 * pi - 4 * qj
                    ec0 = 128 * t0 if t0 > 0 else 0
                    nc.scalar.activation(
                        pt[:, ec0:1024], stp[:, ec0:1024],
                        mybir.ActivationFunctionType.Exp, scale=0.125)
                    for half in range(2):
                        kc = 2 * pi + half
                        t = kc - 4 * qj
                        if 0 <= t <= 3:
                            off = 512 * half + 128 * t
                            nc.vector.tensor_mul(
                                pt[:, off:off + 128],
                                pt[:, off:off + 128], tri_sb)
                    for half in range(2):
                        kc = 2 * pi + half
                        t = kc - 4 * qj
                        c0 = 128 * t if t > 0 else 0
                        nc.tensor.matmul(
                            av[0:HD + 1, c0:512],
                            v_sb[:, kc, h, :],
                            pt[:, 512 * half + c0: 512 * half + 512],
                            start=(kc == 0), stop=(kc == nkc - 1))
                recip = rcpp.tile([1, 512], F32R, tag="rcp", name=f"rcp{qj}_{h}")
                with nc.allow_low_precision("f32r recip for PE broadcast"):
                    nc.vector.reciprocal(recip, av[HD:HD + 1, 0:512])
                rb = ps_sm.tile([P, 512], F32, tag="sm", name=f"rb{qj}_{h}")
                nc.tensor.matmul(rb[0:HD, :], ones_row, recip,
                                 start=True, stop=True)
                rbs = rcpp.tile([HD, 512], F32, tag="rbs", name=f"rbs{qj}_{h}")
                nc.vector.tensor_copy(rbs, rb[0:HD, :])
                c = h // 2
                if h % 2 == 0:
                    nc.vector.tensor_mul(
                        attnT[0:HD, c, q0:q0 + 512],
                        av[0:HD, :], rbs)
                else:
                    tmp = tmpp.tile([HD, 512], F32R, tag="tmp",
                                    name=f"tmp{qj}_{h}")
                    nc.vector.tensor_mul(tmp, av[0:HD, :], rbs)
                    nc.sync.dma_start(
                        out=attnT[HD:P, c, q0:q0 + 512], in_=tmp)

        # ---- phase 6: output projection ----
        for sc in range(NSC):
            y_sb = youtp.tile([P, D], F32, tag="y", name=f"y{sc}")
            for oc in range(2):
                py = ps_sm.tile([P, 512], F32, tag="sm", name=f"py{sc}_{oc}")
                for cc in range(2):
                    nc.tensor.matmul(
                        py[:],
                        attnT[:, cc, sc * P:(sc + 1) * P],
                        wout_sb[:, cc, oc * 512:(oc + 1) * 512],
                        start=(cc == 0), stop=(cc == 1))
                nc.vector.tensor_copy(y_sb[:, oc * 512:(oc + 1) * 512], py[:])
            nc.sync.dma_start(out=y_d.ap()[sc * P:(sc + 1) * P, :], in_=y_sb)

    nc.compile()
    _CACHED["nc"] = nc
    return nc


def _host_inputs(x, W_qkv, b_qkv):
    """Build the 8 per-core input maps."""
    x = np.ascontiguousarray(x, dtype=np.float32)
    tri = (np.arange(P)[None, :] >= np.arange(P)[:, None]).astype(np.float32)
    in_maps = []
    for b in range(B):
        for hg in range(HG):
            c0 = hg * CL
            wqk = np.ascontiguousarray(
                np.concatenate([W_qkv[:, c0:c0 + CL],
                                W_qkv[:, D + c0:D + c0 + CL]], axis=1),
                dtype=np.float32)
            wv = np.ascontiguousarray(W_qkv[:, 2 * D + c0:2 * D + c0 + CL],
                                      dtype=np.float32)
            bqk = np.ascontiguousarray(
                np.concatenate([b_qkv[c0:c0 + CL],
                                b_qkv[D + c0:D + c0 + CL]])
                .reshape(4, P).T, dtype=np.float32)
            in_maps.append({
                "x": x[b], "wqk": wqk, "wv": wv, "wout": None,
                "bqk": bqk, "tri": tri, "idn": np.eye(P, dtype=np.float32),
                "ones": np.ones((P, 68), dtype=np.float32),
            })
    return in_maps


def kernel(x, W_qkv, b_qkv, W_out, b_out):
    x = np.asarray(x, dtype=np.float32)
    W_qkv = np.asarray(W_qkv, dtype=np.float32)
    b_qkv = np.asarray(b_qkv, dtype=np.float32)
    W_out = np.asarray(W_out, dtype=np.float32)
    b_out = np.asarray(b_out, dtype=np.float32)

    nc = _build()
    in_maps = _host_inputs(x, W_qkv, b_qkv)
    for i, m in enumerate(in_maps):
        hg = i % HG
        m["wout"] = np.ascontiguousarray(W_out[hg * CL:(hg + 1) * CL, :],
                                         dtype=np.float32)
    core_ids = list(range(8))
    res = run_bass_kernel_spmd(nc, in_maps, core_ids)
    outs = [r["y"] for r in res.results]
    bv = b_qkv[2 * D:3 * D]
    corr = (bv @ W_out + b_out).astype(np.float32)
    y = np.empty((B, S, D), dtype=np.float32)
    for b in range(B):
        acc = outs[b * HG].astype(np.float32).copy()
        for hg in range(1, HG):
            acc += outs[b * HG + hg]
        y[b] = acc + corr
    return y


# revision 16
# speedup vs baseline: 1.3319x; 1.3319x over previous
"""Causal self-attention Trainium2 kernel.

Reference (full): x[B=2,S=2048,D=1024] @ W_qkv + b_qkv -> 16-head causal
attention -> @ W_out + b_out.

Sharding: 8 cores = (batch b in 0..1) x (head-group hg in 0..3, 4 heads of
hd=64 each). Each core computes a partial output projection for its 4 heads
on its batch; the host sums the 4 head-group partials per batch and adds the
(constant) V-bias correction bv @ W_out and b_out.

Device pipeline per core:
  1. x -> xT via PE transposes (fp32).
  2. QK^T projection (fp32r): qkt[c, s], bias added on eviction.
  3. V projection (fp32r) in natural layout v[s, c] (stored bf16) with a
     ones column appended (produces the softmax denominator during PV).
  4. Per (q-span, head): scores transposed ST[k, q] = KT-slice.T @ QT-slice
     (fp32r), exp on ACT (scale=1/8, bf16 out), triangle mask on diagonal
     blocks only, PV accumulate attnT[c, q] + denominator row (bf16 matmul).
     Causality via partial-N matmuls (skip fully-masked column ranges).
  5. Normalize: evict av PSUM->SBUF fast (frees the bank), broadcast the
     denominator row across partitions with a K=1 fp32r matmul, fast
     reciprocal on 64 lanes, multiply on eviction into attnT (bf16); odd
     heads staged through SBUF and DMA'd to partitions 64..127.
  6. Output projection y[s, o] (bf16 x bf16 -> fp32 PSUM); DMA out.
"""
import numpy as np
import ml_dtypes
from contextlib import ExitStack

import concourse.bacc as bacc
import concourse.tile as tile
from concourse import mybir
from concourse.bass_utils import run_bass_kernel_spmd

F32 = mybir.dt.float32
F32R = mybir.dt.float32r
BF16 = mybir.dt.bfloat16

B = 2
S = 2048
D = 1024
HD = 64
HG = 4            # head-groups (cores per batch)
HPG = 4           # heads per group
CL = HPG * HD     # 256 local head cols per core
P = 128
NSC = S // P      # 16 s-chunks
NDC = D // P      # 8 d-chunks
NQJ = S // 512    # 4 q-spans
NKC = S // P      # 16 k-chunks

_CACHED = {}


def _build():
    if "nc" in _CACHED:
        return _CACHED["nc"]
    nc = bacc.Bacc("TRN2", target_bir_lowering=False, debug=False)

    x_d = nc.dram_tensor("x", [S, D], F32, kind="ExternalInput")
    wqk_d = nc.dram_tensor("wqk", [D, 2 * CL], F32, kind="ExternalInput")
    wv_d = nc.dram_tensor("wv", [D, CL], F32, kind="ExternalInput")
    wout_d = nc.dram_tensor("wout", [CL, D], BF16, kind="ExternalInput")
    bqk_d = nc.dram_tensor("bqk", [P, 4], F32, kind="ExternalInput")
    tri_d = nc.dram_tensor("tri", [P, P], BF16, kind="ExternalInput")
    idn_d = nc.dram_tensor("idn", [P, P], F32, kind="ExternalInput")
    ones_d = nc.dram_tensor("ones", [P, 68], F32, kind="ExternalInput")
    y_d = nc.dram_tensor("y", [S, D], F32, kind="ExternalOutput")

    with tile.TileContext(nc) as tc, ExitStack() as ctx:
        persist = ctx.enter_context(tc.tile_pool(name="persist", bufs=1))
        ptp = ctx.enter_context(tc.tile_pool(name="ptp", bufs=3))
        youtp = ctx.enter_context(tc.tile_pool(name="youtp", bufs=2))
        unp = ctx.enter_context(tc.tile_pool(name="unp", bufs=3))
        rcpp = ctx.enter_context(tc.tile_pool(name="rcpp", bufs=3))
        tmpp = ctx.enter_context(tc.tile_pool(name="tmpp", bufs=2))
        ps_sm = ctx.enter_context(tc.tile_pool(name="ps_sm", bufs=2, space="PSUM"))
        ps_st = ctx.enter_context(tc.tile_pool(name="ps_st", bufs=2, space="PSUM"))
        ps_av = ctx.enter_context(tc.tile_pool(name="ps_av", bufs=2, space="PSUM"))

        # ---- persistent tiles ----
        qkt_sb = persist.tile([P, 4, S], F32R, name="qkt_sb")       # 32KB/part
        v_sb = persist.tile([P, NKC, HPG, HD + 1], BF16, name="v_sb")
        attnT = persist.tile([P, 2, S], BF16, name="attnT")         # 8KB/part
        wout_sb = persist.tile([P, 2, D], BF16, name="wout_sb")
        bqk_sb = persist.tile([P, 4], F32, name="bqk_sb")
        tri_sb = persist.tile([P, P], BF16, name="tri_sb")
        ident = persist.tile([P, P], F32, name="ident")
        ones_sb = persist.tile([P, 68], F32R, name="ones_sb")

        # weights on the ACT HWDGE ring (parallel with x loads on Sync ring)
        nc.scalar.dma_start(out=wout_sb, in_=wout_d.ap()
                            .rearrange("(c p) o -> p c o", p=P))
        # small constants on the SWDGE ring
        nc.gpsimd.dma_start(out=bqk_sb, in_=bqk_d.ap())
        nc.gpsimd.dma_start(out=tri_sb, in_=tri_d.ap())
        nc.gpsimd.dma_start(out=ident, in_=idn_d.ap())
        nc.gpsimd.dma_start(out=ones_sb, in_=ones_d.ap().bitcast(F32R))
        ones_row64 = ones_sb[64:65, 4:4 + HD]

        # fill ones column of V tiles from the DMA'd ones tile
        for sc in range(NSC):
            nc.vector.tensor_copy(v_sb[:, sc, :, HD], ones_sb[:, 0:4])

        # ---- phase 1+2: load x, transpose to xT ----
        with tc.tile_pool(name="wp", bufs=1) as wp:
            wqk_sb = wp.tile([P, NDC, 2 * CL], F32R, name="wqk_sb")
            wv_sb = wp.tile([P, NDC, CL], F32R, name="wv_sb")
            nc.scalar.dma_start(out=wqk_sb, in_=wqk_d.ap().bitcast(F32R)
                                .rearrange("(c p) m -> p c m", p=P))
            nc.scalar.dma_start(out=wv_sb, in_=wv_d.ap().bitcast(F32R)
                                .rearrange("(c p) m -> p c m", p=P))

            with tc.tile_pool(name="xTp", bufs=1) as xTp:
                xT = xTp.tile([P, NDC, S], F32R, name="xT")         # 64KB/part
                with tc.tile_pool(name="xp", bufs=4) as xp:
                    for scq in range(4):
                        xts = []
                        for si in range(4):
                            sc = scq * 4 + si
                            xt = xp.tile([P, D], F32, tag="x", name=f"x{sc}")
                            nc.sync.dma_start(
                                out=xt, in_=x_d.ap()[sc * P:(sc + 1) * P, :])
                            xts.append(xt)
                        for dc in range(NDC):
                            ptr = ps_sm.tile([P, 512], F32, tag="sm",
                                             name=f"ptr{scq}_{dc}")
                            for si in range(4):
                                nc.tensor.transpose(
                                    ptr[:, si * P:(si + 1) * P],
                                    xts[si][:, dc * P:(dc + 1) * P],
                                    ident)
                            nc.vector.tensor_copy(
                                xT[:, dc, scq * 512:(scq + 1) * 512],
                                ptr.bitcast(F32R))

                # ---- phase 3: QK^T projection ----
                for mc in range(4):
                    for ns in range(4):
                        pq = ps_sm.tile([P, 512], F32, tag="sm",
                                        name=f"pq{mc}_{ns}")
                        for kc in range(NDC):
                            nc.tensor.matmul(
                                pq[:],
                                wqk_sb[:, kc, mc * P:(mc + 1) * P],
                                xT[:, kc, ns * 512:(ns + 1) * 512],
                                start=(kc == 0), stop=(kc == NDC - 1))
                        nc.vector.tensor_scalar_add(
                            qkt_sb[:, mc, ns * 512:(ns + 1) * 512],
                            pq[:], bqk_sb[:, mc:mc + 1])

                # ---- phase 4: V projection (natural layout) ----
                for sc in range(NSC):
                    pv = ps_sm.tile([P, CL], F32, tag="sm", name=f"pv{sc}")
                    for kc in range(NDC):
                        nc.tensor.matmul(
                            pv[:],
                            xT[:, kc, sc * P:(sc + 1) * P],
                            wv_sb[:, kc, :],
                            start=(kc == 0), stop=(kc == NDC - 1))
                    nc.vector.tensor_copy(
                        v_sb[:, sc, :, 0:HD],
                        pv.rearrange("p (h d) -> p h d", h=HPG))

        # ---- phase 5: attention per (q-span, head) ----
        for qj in range(NQJ):
            q0 = qj * 512
            nkc = 4 * (qj + 1)
            for h in range(HPG):
                mck, pok = 2 + h // 2, 64 * (h % 2)
                mcq, poq = h // 2, 64 * (h % 2)
                av = ps_av.tile([P, 512], F32, tag="av", name=f"av{qj}_{h}")
                for pi in range(nkc // 2):
                    stp = ps_st.tile([P, 1024], F32, tag="st",
                                     name=f"st{qj}_{h}_{pi}")
                    pt = ptp.tile([P, 1024], BF16, tag="pt",
                                  name=f"pt{qj}_{h}_{pi}")
                    for half in range(2):
                        kc = 2 * pi + half
                        t = kc - 4 * qj
                        c0 = 128 * t if t > 0 else 0
                        nc.tensor.matmul(
                            stp[:, 512 * half + c0: 512 * half + 512],
                            qkt_sb[pok:pok + 64, mck, kc * P:(kc + 1) * P],
                            qkt_sb[poq:poq + 64, mcq, q0 + c0: q0 + 512],
                            start=True, stop=True)
                    t0 = 2 * pi - 4 * qj
                    ec0 = 128 * t0 if t0 > 0 else 0
                    nc.scalar.activation(
                        pt[:, ec0:1024], stp[:, ec0:1024],
                        mybir.ActivationFunctionType.Exp, scale=0.125)
                    for half in range(2):
                        kc = 2 * pi + half
                        t = kc - 4 * qj
                        if 0 <= t <= 3:
                            off = 512 * half + 128 * t
                            nc.vector.tensor_mul(
                                pt[:, off:off + 128],
                                pt[:, off:off + 128], tri_sb)
                    for half in range(2):
                        kc = 2 * pi + half
                        t = kc - 4 * qj
                        c0 = 128 * t if t > 0 else 0
                        nc.tensor.matmul(
                            av[0:HD + 1, c0:512],
                            v_sb[:, kc, h, :],
                            pt[:, 512 * half + c0: 512 * half + 512],
                            start=(kc == 0), stop=(kc == nkc - 1))
                # fast-evict av (frees the PSUM bank), then normalize in SBUF
                un = unp.tile([HD + 1, 512], F32R, tag="un",
                              name=f"un{qj}_{h}")
                nc.vector.tensor_copy(un, av[0:HD + 1, :])
                dnb = ps_sm.tile([P, 512], F32, tag="sm", name=f"dnb{qj}_{h}")
                nc.tensor.matmul(dnb[0:HD, :], ones_row64, un[HD:HD + 1, :],
                                 start=True, stop=True)
                rbs = rcpp.tile([HD, 512], F32, tag="rbs", name=f"rbs{qj}_{h}")
                nc.vector.reciprocal_approx_fast(rbs, dnb[0:HD, :])
                c = h // 2
                if h % 2 == 0:
                    nc.vector.tensor_mul(
                        attnT[0:HD, c, q0:q0 + 512], un[0:HD, :], rbs)
                else:
                    tmp = tmpp.tile([HD, 512], BF16, tag="tmp",
                                    name=f"tmp{qj}_{h}")
                    nc.vector.tensor_mul(tmp, un[0:HD, :], rbs)
                    nc.sync.dma_start(
                        out=attnT[HD:P, c, q0:q0 + 512], in_=tmp)

        # ---- phase 6: output projection ----
        for sc in range(NSC):
            y_sb = youtp.tile([P, D], F32, tag="y", name=f"y{sc}")
            for oc in range(2):
                py = ps_sm.tile([P, 512], F32, tag="sm", name=f"py{sc}_{oc}")
                for cc in range(2):
                    nc.tensor.matmul(
                        py[:],
                        attnT[:, cc, sc * P:(sc + 1) * P],
                        wout_sb[:, cc, oc * 512:(oc + 1) * 512],
                        start=(cc == 0), stop=(cc == 1))
                nc.vector.tensor_copy(y_sb[:, oc * 512:(oc + 1) * 512], py[:])
            nc.sync.dma_start(out=y_d.ap()[sc * P:(sc + 1) * P, :], in_=y_sb)

    nc.compile()
    _CACHED["nc"] = nc
    return nc


def _host_inputs(x, W_qkv, b_qkv):
    """Build the 8 per-core input maps (wout filled in by caller)."""
    x = np.ascontiguousarray(x, dtype=np.float32)
    tri = (np.arange(P)[None, :] >= np.arange(P)[:, None]) \
        .astype(ml_dtypes.bfloat16)
    in_maps = []
    for b in range(B):
        for hg in range(HG):
            c0 = hg * CL
            wqk = np.ascontiguousarray(
                np.concatenate([W_qkv[:, c0:c0 + CL],
                                W_qkv[:, D + c0:D + c0 + CL]], axis=1),
                dtype=np.float32)
            wv = np.ascontiguousarray(W_qkv[:, 2 * D + c0:2 * D + c0 + CL],
                                      dtype=np.float32)
            bqk = np.ascontiguousarray(
                np.concatenate([b_qkv[c0:c0 + CL],
                                b_qkv[D + c0:D + c0 + CL]])
                .reshape(4, P).T, dtype=np.float32)
            in_maps.append({
                "x": x[b], "wqk": wqk, "wv": wv, "wout": None,
                "bqk": bqk, "tri": tri, "idn": np.eye(P, dtype=np.float32),
                "ones": np.ones((P, 68), dtype=np.float32),
            })
    return in_maps


def kernel(x, W_qkv, b_qkv, W_out, b_out):
    x = np.asarray(x, dtype=np.float32)
    W_qkv = np.asarray(W_qkv, dtype=np.float32)
    b_qkv = np.asarray(b_qkv, dtype=np.float32)
    W_out = np.asarray(W_out, dtype=np.float32)
    b_out = np.asarray(b_out, dtype=np.float32)

    nc = _build()
    in_maps = _host_inputs(x, W_qkv, b_qkv)
    for i, m in enumerate(in_maps):
        hg = i % HG
        m["wout"] = np.ascontiguousarray(
            W_out[hg * CL:(hg + 1) * CL, :].astype(ml_dtypes.bfloat16))
    core_ids = list(range(8))
    res = run_bass_kernel_spmd(nc, in_maps, core_ids)
    outs = [r["y"] for r in res.results]
    bv = b_qkv[2 * D:3 * D]
    corr = (bv @ W_out + b_out).astype(np.float32)
    y = np.empty((B, S, D), dtype=np.float32)
    for b in range(B):
        acc = outs[b * HG].astype(np.float32).copy()
        for hg in range(1, HG):
            acc += outs[b * HG + hg]
        y[b] = acc + corr
    return y


# revision 18
# speedup vs baseline: 1.7840x; 1.3395x over previous
"""Causal self-attention Trainium2 kernel.

Reference (full): x[B=2,S=2048,D=1024] @ W_qkv + b_qkv -> 16-head causal
attention -> @ W_out + b_out.

Sharding: 8 cores = (batch b in 0..1) x (head-group hg in 0..3, 4 heads of
hd=64 each). Each core computes a partial output projection for its 4 heads
on its batch; the host sums the 4 head-group partials per batch and adds the
(constant) V-bias correction bv @ W_out and b_out.

Device pipeline per core (data path in fp16 -> 1 cycle/row matmuls; all
accumulation in fp32 PSUM; softmax denominator kept in fp32/f32r because it
can exceed the fp16 range):
  1. x (fp16) -> xT via PE transposes.
  2+3+4 interleaved per q-span to keep the PE activity monitor warm
  (low-utilization K=64 attention matmuls alone get half-clocked):
     QK^T projection slice, V projection slice, per-head attention
     (scores transposed ST[k,q], exp on ACT with 1/8 scale, triangle mask
     on diagonal blocks, PV accumulate attnT + denominator row via a ones
     column in V, causality via partial-N matmuls), then the output
     projection for the finished q-span.
  Normalization: fast PSUM evict, K=1 f32r matmul broadcasts the
  denominator row, reciprocal_approx_fast, multiply on eviction; odd heads
  staged through SBUF and DMA'd to partitions 64..127.
"""
import numpy as np
from contextlib import ExitStack

import concourse.bacc as bacc
import concourse.tile as tile
from concourse import mybir
from concourse.bass_utils import run_bass_kernel_spmd

F32 = mybir.dt.float32
F32R = mybir.dt.float32r
F16 = mybir.dt.float16

B = 2
S = 2048
D = 1024
HD = 64
HG = 4            # head-groups (cores per batch)
HPG = 4           # heads per group
CL = HPG * HD     # 256 local head cols per core
P = 128
NSC = S // P      # 16 s-chunks
NDC = D // P      # 8 d-chunks
NQJ = S // 512    # 4 q-spans
NKC = S // P      # 16 k-chunks

_CACHED = {}


def _build():
    if "nc" in _CACHED:
        return _CACHED["nc"]
    nc = bacc.Bacc("TRN2", target_bir_lowering=False, debug=False)

    x_d = nc.dram_tensor("x", [S, D], F16, kind="ExternalInput")
    wqk_d = nc.dram_tensor("wqk", [D, 2 * CL], F16, kind="ExternalInput")
    wv_d = nc.dram_tensor("wv", [D, CL], F16, kind="ExternalInput")
    wout_d = nc.dram_tensor("wout", [CL, D], F16, kind="ExternalInput")
    bqk_d = nc.dram_tensor("bqk", [P, 4], F32, kind="ExternalInput")
    tri_d = nc.dram_tensor("tri", [P, P], F16, kind="ExternalInput")
    idn_d = nc.dram_tensor("idn", [P, P], F16, kind="ExternalInput")
    ones_d = nc.dram_tensor("ones", [P, 68], F32, kind="ExternalInput")
    y_d = nc.dram_tensor("y", [S, D], F32, kind="ExternalOutput")

    with tile.TileContext(nc) as tc, ExitStack() as ctx:
        persist = ctx.enter_context(tc.tile_pool(name="persist", bufs=1))
        ptp = ctx.enter_context(tc.tile_pool(name="ptp", bufs=3))
        youtp = ctx.enter_context(tc.tile_pool(name="youtp", bufs=2))
        unp = ctx.enter_context(tc.tile_pool(name="unp", bufs=2))
        rcpp = ctx.enter_context(tc.tile_pool(name="rcpp", bufs=2))
        tmpp = ctx.enter_context(tc.tile_pool(name="tmpp", bufs=2))
        ps_sm = ctx.enter_context(tc.tile_pool(name="ps_sm", bufs=2, space="PSUM"))
        ps_st = ctx.enter_context(tc.tile_pool(name="ps_st", bufs=2, space="PSUM"))
        ps_av = ctx.enter_context(tc.tile_pool(name="ps_av", bufs=2, space="PSUM"))

        # ---- persistent tiles ----
        qkt_sb = persist.tile([P, 4, S], F16, name="qkt_sb")        # 16KB/part
        v_sb = persist.tile([P, NKC, HPG, HD + 1], F16, name="v_sb")
        attnT = persist.tile([P, 2, S], F16, name="attnT")          # 8KB/part
        wout_sb = persist.tile([P, 2, D], F16, name="wout_sb")
        wqk_sb = persist.tile([P, NDC, 2 * CL], F16, name="wqk_sb")
        wv_sb = persist.tile([P, NDC, CL], F16, name="wv_sb")
        bqk_sb = persist.tile([P, 4], F32, name="bqk_sb")
        tri_sb = persist.tile([P, P], F16, name="tri_sb")
        ident = persist.tile([P, P], F16, name="ident")
        ones_sb = persist.tile([P, 68], F32R, name="ones_sb")

        # weights on the ACT HWDGE ring (parallel with x loads on Sync ring)
        nc.scalar.dma_start(out=wqk_sb, in_=wqk_d.ap()
                            .rearrange("(c p) m -> p c m", p=P))
        nc.scalar.dma_start(out=wv_sb, in_=wv_d.ap()
                            .rearrange("(c p) m -> p c m", p=P))
        nc.scalar.dma_start(out=wout_sb, in_=wout_d.ap()
                            .rearrange("(c p) o -> p c o", p=P))
        # small constants on the SWDGE ring
        nc.gpsimd.dma_start(out=bqk_sb, in_=bqk_d.ap())
        nc.gpsimd.dma_start(out=tri_sb, in_=tri_d.ap())
        nc.gpsimd.dma_start(out=ident, in_=idn_d.ap())
        nc.gpsimd.dma_start(out=ones_sb, in_=ones_d.ap().bitcast(F32R))
        ones_row64 = ones_sb[64:65, 4:4 + HD]

        # fill ones column of V tiles
        for sc in range(NSC):
            nc.vector.tensor_copy(v_sb[:, sc, :, HD], ones_sb[:, 0:4])

        # ---- phase 1+2: load x, transpose to xT ----
        with tc.tile_pool(name="xTp", bufs=1) as xTp:
            xT = xTp.tile([P, NDC, S], F16, name="xT")              # 32KB/part
            with tc.tile_pool(name="xp", bufs=8) as xp:
                for scq in range(4):
                    xts = []
                    for si in range(4):
                        sc = scq * 4 + si
                        xt = xp.tile([P, D], F16, tag="x", name=f"x{sc}")
                        nc.sync.dma_start(
                            out=xt, in_=x_d.ap()[sc * P:(sc + 1) * P, :])
                        xts.append(xt)
                    for dc in range(NDC):
                        ptr = ps_sm.tile([P, 512], F16, tag="sm",
                                         name=f"ptr{scq}_{dc}")
                        for si in range(4):
                            nc.tensor.transpose(
                                ptr[:, si * P:(si + 1) * P],
                                xts[si][:, dc * P:(dc + 1) * P],
                                ident)
                        nc.vector.tensor_copy(
                            xT[:, dc, scq * 512:(scq + 1) * 512], ptr)

            # ---- phases 3..6 interleaved per q-span ----
            for qj in range(NQJ):
                q0 = qj * 512
                nkc = 4 * (qj + 1)

                # QK^T projection slice ns=qj (full-util work)
                for mc in range(4):
                    pq = ps_sm.tile([P, 512], F32, tag="sm",
                                    name=f"pq{mc}_{qj}")
                    for kc in range(NDC):
                        nc.tensor.matmul(
                            pq[:],
                            wqk_sb[:, kc, mc * P:(mc + 1) * P],
                            xT[:, kc, q0:q0 + 512],
                            start=(kc == 0), stop=(kc == NDC - 1))
                    nc.vector.tensor_scalar_add(
                        qkt_sb[:, mc, q0:q0 + 512], pq[:],
                        bqk_sb[:, mc:mc + 1])

                # V projection slices sc = 4qj .. 4qj+3
                for si in range(4):
                    sc = 4 * qj + si
                    pv = ps_sm.tile([P, CL], F32, tag="sm", name=f"pv{sc}")
                    for kc in range(NDC):
                        nc.tensor.matmul(
                            pv[:],
                            xT[:, kc, sc * P:(sc + 1) * P],
                            wv_sb[:, kc, :],
                            start=(kc == 0), stop=(kc == NDC - 1))
                    nc.vector.tensor_copy(
                        v_sb[:, sc, :, 0:HD],
                        pv.rearrange("p (h d) -> p h d", h=HPG))

                # attention for this q-span
                for h in range(HPG):
                    mck, pok = 2 + h // 2, 64 * (h % 2)
                    mcq, poq = h // 2, 64 * (h % 2)
                    av = ps_av.tile([P, 512], F32, tag="av",
                                    name=f"av{qj}_{h}")
                    for pi in range(nkc // 2):
                        stp = ps_st.tile([P, 1024], F32, tag="st",
                                         name=f"st{qj}_{h}_{pi}")
                        pt = ptp.tile([P, 1024], F16, tag="pt",
                                      name=f"pt{qj}_{h}_{pi}")
                        for half in range(2):
                            kc = 2 * pi + half
                            t = kc - 4 * qj
                            c0 = 128 * t if t > 0 else 0
                            nc.tensor.matmul(
                                stp[:, 512 * half + c0: 512 * half + 512],
                                qkt_sb[pok:pok + 64, mck, kc * P:(kc + 1) * P],
                                qkt_sb[poq:poq + 64, mcq, q0 + c0: q0 + 512],
                                start=True, stop=True)
                        t0 = 2 * pi - 4 * qj
                        ec0 = 128 * t0 if t0 > 0 else 0
                        nc.scalar.activation(
                            pt[:, ec0:1024], stp[:, ec0:1024],
                            mybir.ActivationFunctionType.Exp, scale=0.125)
                        for half in range(2):
                            kc = 2 * pi + half
                            t = kc - 4 * qj
                            if 0 <= t <= 3:
                                off = 512 * half + 128 * t
                                nc.vector.tensor_mul(
                                    pt[:, off:off + 128],
                                    pt[:, off:off + 128], tri_sb)
                        for half in range(2):
                            kc = 2 * pi + half
                            t = kc - 4 * qj
                            c0 = 128 * t if t > 0 else 0
                            nc.tensor.matmul(
                                av[0:HD + 1, c0:512],
                                v_sb[:, kc, h, :],
                                pt[:, 512 * half + c0: 512 * half + 512],
                                start=(kc == 0), stop=(kc == nkc - 1))
                    # fast-evict av, then normalize in SBUF
                    un = unp.tile([HD + 1, 512], F32R, tag="un",
                                  name=f"un{qj}_{h}")
                    nc.vector.tensor_copy(un, av[0:HD + 1, :])
                    dnb = ps_sm.tile([P, 512], F32, tag="sm",
                                     name=f"dnb{qj}_{h}")
                    nc.tensor.matmul(dnb[0:HD, :], ones_row64,
                                     un[HD:HD + 1, :], start=True, stop=True)
                    rbs = rcpp.tile([HD, 512], F32, tag="rbs",
                                    name=f"rbs{qj}_{h}")
                    nc.vector.reciprocal_approx_fast(rbs, dnb[0:HD, :])
                    c = h // 2
                    if h % 2 == 0:
                        nc.vector.tensor_mul(
                            attnT[0:HD, c, q0:q0 + 512], un[0:HD, :], rbs)
                    else:
                        tmp = tmpp.tile([HD, 512], F16, tag="tmp",
                                        name=f"tmp{qj}_{h}")
                        nc.vector.tensor_mul(tmp, un[0:HD, :], rbs)
                        nc.sync.dma_start(
                            out=attnT[HD:P, c, q0:q0 + 512], in_=tmp)

                # output projection for this q-span (full-util work)
                for si in range(4):
                    sc = 4 * qj + si
                    y_sb = youtp.tile([P, D], F32, tag="y", name=f"y{sc}")
                    for oc in range(2):
                        py = ps_sm.tile([P, 512], F32, tag="sm",
                                        name=f"py{sc}_{oc}")
                        for cc in range(2):
                            nc.tensor.matmul(
                                py[:],
                                attnT[:, cc, sc * P:(sc + 1) * P],
                                wout_sb[:, cc, oc * 512:(oc + 1) * 512],
                                start=(cc == 0), stop=(cc == 1))
                        nc.vector.tensor_copy(
                            y_sb[:, oc * 512:(oc + 1) * 512], py[:])
                    nc.sync.dma_start(
                        out=y_d.ap()[sc * P:(sc + 1) * P, :], in_=y_sb)

    nc.compile()
    _CACHED["nc"] = nc
    return nc


def _host_inputs(x, W_qkv, b_qkv):
    """Build the 8 per-core input maps (wout filled in by caller)."""
    x16 = np.asarray(x, dtype=np.float16)
    tri = (np.arange(P)[None, :] >= np.arange(P)[:, None]).astype(np.float16)
    in_maps = []
    for b in range(B):
        for hg in range(HG):
            c0 = hg * CL
            wqk = np.ascontiguousarray(
                np.concatenate([W_qkv[:, c0:c0 + CL],
                                W_qkv[:, D + c0:D + c0 + CL]], axis=1)
                .astype(np.float16))
            wv = np.ascontiguousarray(
                W_qkv[:, 2 * D + c0:2 * D + c0 + CL].astype(np.float16))
            bqk = np.ascontiguousarray(
                np.concatenate([b_qkv[c0:c0 + CL],
                                b_qkv[D + c0:D + c0 + CL]])
                .reshape(4, P).T, dtype=np.float32)
            in_maps.append({
                "x": x16[b], "wqk": wqk, "wv": wv, "wout": None,
                "bqk": bqk, "tri": tri, "idn": np.eye(P, dtype=np.float16),
                "ones": np.ones((P, 68), dtype=np.float32),
            })
    return in_maps


def kernel(x, W_qkv, b_qkv, W_out, b_out):
    x = np.asarray(x, dtype=np.float32)
    W_qkv = np.asarray(W_qkv, dtype=np.float32)
    b_qkv = np.asarray(b_qkv, dtype=np.float32)
    W_out = np.asarray(W_out, dtype=np.float32)
    b_out = np.asarray(b_out, dtype=np.float32)

    nc = _build()
    in_maps = _host_inputs(x, W_qkv, b_qkv)
    for i, m in enumerate(in_maps):
        hg = i % HG
        m["wout"] = np.ascontiguousarray(
            W_out[hg * CL:(hg + 1) * CL, :].astype(np.float16))
    core_ids = list(range(8))
    res = run_bass_kernel_spmd(nc, in_maps, core_ids)
    outs = [r["y"] for r in res.results]
    bv = b_qkv[2 * D:3 * D]
    corr = (bv @ W_out + b_out).astype(np.float32)
    y = np.empty((B, S, D), dtype=np.float32)
    for b in range(B):
        acc = outs[b * HG].astype(np.float32).copy()
        for hg in range(1, HG):
            acc += outs[b * HG + hg]
        y[b] = acc + corr
    return y
